# Initial kernel scaffold
#
"""Trainium2 Bass kernel for a batch of biquad IIR filters (Butterworth low-pass).

Reference computation (per waveform row, fp32):
    u[t] = (b0*x[t] + b1*x[t-1] + b2*x[t-2]) / a0     (causal FIR, zero pad)
    y[t] = u[t] - (a2/a0)*y[t-2]                       (feedback; a1 == 0)

Since a1 == 0 the feedback splits into two independent lag-1 recurrences
over the even/odd sample streams, each mapping onto the DVE
TensorTensorScan instruction (state = c*state + u along the free dim).

Engine assignment (per 64-row shard on each of 8 cores):
  Pool - first FIR stage w = x + x[-1], one tensor_tensor per DMA tile
         (b1 == 2*b0 and b2 == b0 make the FIR (1+z^-1)^2 * b0).
  PE   - second FIR stage + scale as accumulating scaled-identity matmuls
         into PSUM: u = (b0/a0 I)w + (b0/a0 I)w[-1]. Both taps share one
         stationary weight; time shifts are free in the moving-operand AP.
         (Generic-b fallback: 3 taps straight from x with 3 weights.)
  DVE  - two parity scans per 1024-col chunk, reading u strided directly
         from PSUM and writing y interleaved into the output SBUF tile.
         Scans chain across chunks via initial=prev_out[:, -1:], so no
         per-chunk halo warm-up is needed and the recurrence is exact.
  SP   - input DMA issue;  Act - output DMA issue (separate queues so an
         output stall never blocks input prefetch).

Sharding: batch 512 -> 64 rows per core (pure data parallelism). Per core
the (64, 65536) shard is viewed as 128 partitions x 32768 columns: each row
is split into two halves processed in parallel partitions. The h=1 halves
start mid-stream, so their scan state is seeded once by a 32-sample warm-up
scan over u(x[THALF-32:THALF]) (c = -a2/a0, |c|=0.17: c^16 ~ 6e-13, far
below fp32 noise). h=0 rows see zero warm-up input, keeping their initial
state exactly 0.
"""

import numpy as np

# Problem geometry (hardcoded per the grading contract).
N_CORES = 8
BATCH = 512
T = 65536
ROWS = BATCH // N_CORES          # 64 rows per core
THALF = T // 2                   # each row is split into 2 half-rows
P = 128                          # SBUF partitions = 2 * ROWS
CU = 1024                        # PSUM chunk columns (2 banks)
N_CHUNKS = THALF // CU           # 32
TILE_CHUNKS = 4                  # chunks per DMA tile
XPAD = 40                        # left pad: 2 FIR history + 38 warm-up cols
XTW = TILE_CHUNKS * CU + XPAD    # 4136 input tile cols
YTW = TILE_CHUNKS * CU           # 4096 output tile cols
N_TILES = N_CHUNKS // TILE_CHUNKS  # 8
GW = 512                         # matmul group cols (one PSUM bank)
WARM = 32                        # warm-up samples for h=1 scan state
XD = 2                           # input dma_starts per half-row (8KB descs)


def _build_program(c: float, r01: float, r12: float, b2p: float,
                   two_tap: bool):
    """Build + compile the per-core Bass program.

    c = -a2/a0, r01 = b0/b1, r12 = b1/b2, b2p = b2/a0 (warm-up Horner).
    two_tap: b == b0*(1, 2, 1), enabling the Pool w-stage + 2-tap PE FIR.
    """
    import concourse.bacc as bacc
    import concourse.mybir as mybir
    import concourse.tile as tile

    dt = mybir.dt.float32
    ALU = mybir.AluOpType

    nc = bacc.Bacc("TRN2", target_bir_lowering=False, debug=False)
    x = nc.dram_tensor("x", [N_TILES, P, XTW], dt, kind="ExternalInput")
    w0 = nc.dram_tensor("w0", [P, P], dt, kind="ExternalInput")  # b0/a0 * I
    w1 = nc.dram_tensor("w1", [P, P], dt, kind="ExternalInput")  # b1/a0 * I
    w2 = nc.dram_tensor("w2", [P, P], dt, kind="ExternalInput")  # b2/a0 * I
    y = nc.dram_tensor("y", [N_TILES, P, YTW], dt, kind="ExternalOutput")

    with tile.TileContext(nc) as tc:
        with (
            tc.tile_pool(name="const", bufs=1) as cpool,
            tc.tile_pool(name="xin", bufs=3) as xpool,
            tc.tile_pool(name="wfir", bufs=2) as wpool,
            tc.tile_pool(name="yout", bufs=3) as ypool,
            tc.tile_pool(name="ps", bufs=4, space="PSUM") as psum,
            tc.tile_pool(name="vst", bufs=2) as vpool,
        ):
            wt = []
            for i, w in enumerate((w0, w1, w2)):
                t_ = cpool.tile([P, P], dt, tag=f"w{i}")
                nc.scalar.dma_start(out=t_[:], in_=w[:, :])
                wt.append(t_)
            ctile = cpool.tile([P, 1], dt, tag="ctile")
            nc.vector.memset(ctile[:], c)
            cbca = ctile[:, 0:1].to_broadcast([P, CU // 2])
            cbcw = ctile[:, 0:1].to_broadcast([P, WARM // 2])
            uw = cpool.tile([P, WARM], dt, tag="uw")
            tw = cpool.tile([P, WARM + 1], dt, tag="tw")
            zw = cpool.tile([P, WARM], dt, tag="zw")

            prev_yt = None
            prev_zv = None
            for m in range(N_TILES):
                lo = m * YTW  # first output col of this tile (per half-row)
                xt = xpool.tile([P, XTW], dt, tag="xt")
                # Input is pre-tiled host-side: xd[m] is exactly this tile
                # (zero padding for the h=0 sequence start baked in), so
                # input descriptors are sequential in DRAM. Tile 0 is
                # loaded in chunk-sized segments so compute starts early.
                nseg = TILE_CHUNKS if m == 0 else 1
                seg = [0] + [XPAD + YTW * (i + 1) // nseg for i in range(nseg)]
                for i in range(nseg):
                    a_, b_ = seg[i], seg[i + 1]
                    nc.sync.dma_start(out=xt[:, a_:b_], in_=x[m, :, a_:b_])

                if two_tap:
                    # w-tile piece p covers w-indices [p*W0, p*W0 + W0 + 1)
                    # where w-index i holds w(lo - 1 + i); computed from xt
                    # cols [XPAD-2+p*W0, ...). Quarters on tile 0 (so the
                    # pipeline starts after one input segment), else halves.
                    npiece = TILE_CHUNKS if m == 0 else 2
                    W0 = YTW // npiece
                    wmap = []
                    for p_ in range(npiece):
                        wp = wpool.tile([P, W0 + 1], dt,
                                        tag=f"w{npiece}_{p_}")
                        o_ = XPAD - 2 + p_ * W0
                        nc.gpsimd.tensor_tensor(
                            out=wp[:, 0 : W0 + 1],
                            in0=xt[:, o_ : o_ + W0 + 1],
                            in1=xt[:, o_ + 1 : o_ + W0 + 2],
                            op=ALU.add)
                        for _ in range(W0 // CU):
                            wmap.append((wp, p_ * W0))

                yt = ypool.tile([P, YTW], dt, tag="yt")
                if m == 0:
                    # Warm-up: u over the WARM cols before each stream start
                    # (zeros for h=0, tail of h=0 for h=1), then 16-step
                    # parity scans to seed the chunk-0 scan states exactly.
                    if two_tap:
                        # x is host-prescaled by b0/a0, so u = w + w[-1]
                        # with w = x + x[-1]: two adds.
                        # tww[i] = w(t0 - 33 + i), i in [0, WARM+1)
                        nc.vector.tensor_tensor(
                            out=tw[:, 0 : WARM + 1],
                            in0=xt[:, XPAD - WARM - 2 : XPAD - 1],
                            in1=xt[:, XPAD - WARM - 1 : XPAD],
                            op=ALU.add)
                        nc.vector.tensor_tensor(
                            out=uw[:], in0=tw[:, 0:WARM],
                            in1=tw[:, 1 : WARM + 1], op=ALU.add)
                    else:
                        wbase = XPAD - WARM
                        nc.vector.scalar_tensor_tensor(
                            out=tw[:, 0:WARM],
                            in0=xt[:, wbase : wbase + WARM], scalar=r01,
                            in1=xt[:, wbase - 1 : wbase + WARM - 1],
                            op0=ALU.mult, op1=ALU.add)
                        nc.vector.scalar_tensor_tensor(
                            out=uw[:], in0=tw[:, 0:WARM], scalar=r12,
                            in1=xt[:, wbase - 2 : wbase + WARM - 2],
                            op0=ALU.mult, op1=ALU.add)
                        nc.vector.tensor_scalar_mul(uw[:], uw[:], b2p)
                    for par in range(2):
                        nc.vector.tensor_tensor_scan(
                            out=zw[:, par * (WARM // 2) : (par + 1) * (WARM // 2)],
                            data0=cbcw,
                            data1=uw[:, par : WARM : 2],
                            initial=0.0, op0=ALU.mult, op1=ALU.add)

                for s in range(TILE_CHUNKS):
                    k = TILE_CHUNKS * m + s
                    dve_v = two_tap and s == TILE_CHUNKS - 1
                    HC = CU // 2
                    if dve_v:
                        # Pool deinterleaves this chunk's u = w + w[-1] into
                        # compact parity halves (stride-2 Pool reads are full
                        # speed); scans run unit/unit into a compact z tile
                        # and Act interleaves into yt. (Scans with unit data1
                        # + strided out are miscompiled; unit/unit is safe.)
                        wh, wbase_ = wmap[s]
                        ao = s * CU - wbase_
                        ut = vpool.tile([P, CU], dt, tag="v")
                        zv = vpool.tile([P, CU], dt, tag="zv")
                        nc.gpsimd.tensor_tensor(
                            out=ut[:, 0:HC],
                            in0=wh[:, ao : ao + CU : 2],
                            in1=wh[:, ao + 1 : ao + CU : 2],
                            op=ALU.add)
                        nc.gpsimd.tensor_tensor(
                            out=ut[:, HC:CU],
                            in0=wh[:, ao + 1 : ao + CU + 1 : 2],
                            in1=wh[:, ao + 2 : ao + CU + 1 : 2],
                            op=ALU.add)
                    else:
                        ut = psum.tile([P, CU], dt, tag="u")
                    for g in range(CU // GW if not dve_v else 0):
                        a = s * CU + g * GW  # u col within tile
                        if two_tap:
                            # u[t] = w[t] + w[t-1] (x host-prescaled by
                            # b0/a0, weights = identity);
                            # w-index i = t - lo + 1, from this chunk's piece
                            wh, wbase_ = wmap[s]
                            ao = a - wbase_
                            nc.tensor.matmul(
                                ut[:, g * GW : g * GW + GW], wt[0][:],
                                wh[:, ao + 1 : ao + GW + 1],
                                start=True, stop=False)
                            nc.tensor.matmul(
                                ut[:, g * GW : g * GW + GW], wt[0][:],
                                wh[:, ao : ao + GW],
                                start=False, stop=True)
                        else:
                            base = XPAD + a
                            nc.tensor.matmul(
                                ut[:, g * GW : g * GW + GW], wt[0][:],
                                xt[:, base : base + GW],
                                start=True, stop=False)
                            nc.tensor.matmul(
                                ut[:, g * GW : g * GW + GW], wt[1][:],
                                xt[:, base - 1 : base + GW - 1],
                                start=False, stop=False)
                            nc.tensor.matmul(
                                ut[:, g * GW : g * GW + GW], wt[2][:],
                                xt[:, base - 2 : base + GW - 2],
                                start=False, stop=True)

                    off = s * CU
                    for par in range(2):
                        if k == 0:
                            init = zw[:, (par + 1) * (WARM // 2) - 1
                                      : (par + 1) * (WARM // 2)]
                        elif s == 0:
                            # every tile-start follows a v-chunk (k%4==0):
                            # chain from its compact z tile
                            init = prev_zv[:, (par + 1) * HC - 1
                                           : (par + 1) * HC]
                        else:
                            init = yt[:, off - 2 + par : off - 1 + par]
                        if dve_v:
                            nc.vector.tensor_tensor_scan(
                                out=zv[:, par * HC : (par + 1) * HC],
                                data0=cbca,
                                data1=ut[:, par * HC : (par + 1) * HC],
                                initial=init,
                                op0=ALU.mult, op1=ALU.add,
                            )
                            nc.scalar.copy(
                                yt[:, off + par : off + CU : 2],
                                zv[:, par * HC : (par + 1) * HC])
                        else:
                            nc.vector.tensor_tensor_scan(
                                out=yt[:, off + par : off + CU : 2],
                                data0=cbca,
                                data1=ut[:, par : CU : 2],
                                initial=init,
                                op0=ALU.mult, op1=ALU.add,
                            )
                    if dve_v:
                        prev_zv = zv
                    if m == N_TILES - 1 or s % 2 == 1:
                        if m == N_TILES - 1:
                            ha, hb = s * CU, (s + 1) * CU
                        else:
                            ha, hb = (s - 1) * CU, (s + 1) * CU
                        nc.scalar.dma_start(
                            out=y[m, :, ha:hb], in_=yt[:, ha:hb])

                prev_yt = yt

    nc.compile()
    return nc


_CACHE: dict = {}


def _get_program(b, a):
    b0, b1, b2 = (float(v) for v in np.asarray(b, dtype=np.float64))
    a0, a1, a2 = (float(v) for v in np.asarray(a, dtype=np.float64))
    assert a1 == 0.0, "kernel exploits a1 == 0 (even/odd stream decoupling)"
    assert b1 != 0.0 and b2 != 0.0, "warm-up Horner needs nonzero b1, b2"
    key = (b0, b1, b2, a0, a2)
    if key not in _CACHE:
        c = np.float32(-np.float32(a2) / np.float32(a0))
        r01 = np.float32(np.float32(b0) / np.float32(b1))
        r12 = np.float32(np.float32(b1) / np.float32(b2))
        b2p = np.float32(np.float32(b2) / np.float32(a0))
        two_tap = (b1 == 2.0 * b0) and (b2 == b0)
        _CACHE[key] = _build_program(
            float(c), float(r01), float(r12), float(b2p), two_tap
        )
    return _CACHE[key]


def _weights(b, a):
    b0, b1, b2 = (np.float32(v) for v in np.asarray(b, dtype=np.float32))
    a0 = np.float32(np.asarray(a, dtype=np.float32)[0])
    eye = np.eye(P, dtype=np.float32)
    return (
        np.ascontiguousarray(eye * np.float32(b0 / a0)),
        np.ascontiguousarray(eye * np.float32(b1 / a0)),
        np.ascontiguousarray(eye * np.float32(b2 / a0)),
    )


def run(x, b, a, trace: bool = False):
    """Run the kernel on the full (512, 65536) input; returns (y, exec_time_ns)."""
    from concourse.bass_utils import run_bass_kernel_spmd

    x = np.asarray(x, dtype=np.float32)
    assert x.shape == (BATCH, T), x.shape
    nc = _get_program(b, a)
    w0, w1, w2 = _weights(b, a)
    bf = np.asarray(b, dtype=np.float32)
    af = np.asarray(a, dtype=np.float32)
    two_tap = (float(bf[1]) == 2.0 * float(bf[0])) and float(bf[2]) == float(bf[0])
    b0p = np.float32(bf[0] / af[0])
    if two_tap:
        w0 = np.ascontiguousarray(np.eye(P, dtype=np.float32))

    shards = x.reshape(N_CORES, ROWS, T)
    in_maps = []
    for i in range(N_CORES):
        sh = shards[i]
        xd = np.zeros((N_TILES, P, XTW), dtype=np.float32)
        for m in range(N_TILES):
            lo = m * YTW
            if m == 0:
                xd[0, 0:ROWS, XPAD:] = sh[:, 0:YTW]
                xd[0, ROWS:P, :] = sh[:, THALF - XPAD : THALF + YTW]
            else:
                xd[m, 0:ROWS, :] = sh[:, lo - XPAD : lo + YTW]
                xd[m, ROWS:P, :] = sh[:, THALF + lo - XPAD : THALF + lo + YTW]
        if two_tap:
            xd *= b0p
        in_maps.append({"x": xd, "w0": w0, "w1": w1, "w2": w2})
    res = run_bass_kernel_spmd(nc, in_maps, list(range(N_CORES)), trace=trace)
    out = np.empty((BATCH, T), dtype=np.float32)
    for i in range(N_CORES):
        yd = res.results[i]["y"]            # (N_TILES, P, YTW)
        blk = out[i * ROWS : (i + 1) * ROWS]
        h0 = yd[:, 0:ROWS, :]               # (N_TILES, ROWS, YTW)
        h1 = yd[:, ROWS:P, :]
        blk[:, 0:THALF] = h0.transpose(1, 0, 2).reshape(ROWS, THALF)
        blk[:, THALF:T] = h1.transpose(1, 0, 2).reshape(ROWS, THALF)
    return out, res.exec_time_ns


def kernel(x, b, a):
    out, _ = run(x, b, a, trace=False)
    return out



# revision 19
# speedup vs baseline: 2.0135x; 2.0135x over previous
"""Trainium2 Bass kernel: batch biquad IIR as a truncated-FIR banded matmul.

The reference IIR y[t] = sum_m b[m] x[t-m]/a0 - sum_n a[n]/a0 y[t-n] has a
fast-decaying impulse response for this filter (poles at |z| = sqrt(0.1716)),
so y is computed exactly (to below-fp32-noise truncation, |h| < 1e-9) as a
J~25-tap FIR of x. This removes the time recurrence entirely: no DVE scan
(3 cycles/element serial — 82us/core for this shape), no feedback state.

Layout: each waveform row (65536 samples) is reshaped host-side to a
[128, 512] segment matrix X (partition k holds samples 128*f + k). Then

    y(128f + i) = sum_k W1[k, i] X[k, f] + sum_k W2[k, i] X[k, f-1]

with W1[k,i] = h[i-k] (banded lower Toeplitz) and W2[k,i] = h[i+128-k]
(corner band), i.e. TWO accumulating PE matmuls per 512-column group with
stationary weights. Rows are concatenated along the free axis; the one
cross-row halo column per row start is corrected on the host (the leak is
linear and only touches the first J-1 samples of each row).

Engine budget per core (8 tiles x 4096 cols): PE ~30us of fp16 matmuls
(1 cyc/col), PSUM->SBUF cast copies split across Scalar/DVE/Pool ~15us
each, DMA 2x8.4MB fp16 ~47us -> DMA-bound.

Sharding: pure data parallel, 64 rows per core on 8 cores. I/O is fp16
(x cast host-side, y cast back): halves HBM traffic; error ~1e-3 absolute
(~2.5e-4 of output scale) vs the 2e-2 harness gate.
"""

import numpy as np

# Problem geometry (hardcoded per the grading contract).
N_CORES = 8
BATCH = 512
T = 65536
ROWS = BATCH // N_CORES          # 64 rows per core
SEG = 128                        # samples per segment column (= partitions)
F = T // SEG                     # 512 segment columns per row
COLS = ROWS * F                  # 32768 free-axis columns per core
CW = 4096                        # tile width (free cols); 8 groups of 512
NT = COLS // CW                  # 8 tiles
GW = 512                         # matmul group cols (one PSUM bank)
HW_ = 1024                       # copy granularity (two banks)

# I/O + matmul precision mode: "fp16" | "bf16" | "fp32r"
IO_MODE = "fp16"


def _fir_taps(b, a, tol=1e-9, max_taps=120):
    """Impulse response of the IIR filter, truncated to J taps (float64)."""
    b = np.asarray(b, np.float64)
    a = np.asarray(a, np.float64)
    L = 256
    h = np.zeros(L)
    for t in range(L):
        acc = b[t] if t < len(b) else 0.0
        for n in range(1, len(a)):
            if t - n >= 0:
                acc -= a[n] * h[t - n]
        h[t] = acc / a[0]
    mag = np.abs(h)
    keep = np.nonzero(mag > tol * mag.max())[0]
    J = int(keep.max()) + 1
    assert J <= max_taps, f"impulse response too long for FIR approach: {J}"
    assert J <= SEG, J
    return h[:J]


def _np_dt(mode):
    return {"fp16": np.float16, "bf16": None, "fp32r": np.float32}[mode]


def _build_program(mode):
    import concourse.bacc as bacc
    import concourse.mybir as mybir
    import concourse.tile as tile

    dt_io = {
        "fp16": mybir.dt.float16,
        "bf16": mybir.dt.bfloat16,
        "fp32r": mybir.dt.float32r,
    }[mode]
    f32 = mybir.dt.float32

    nc = bacc.Bacc("TRN2", target_bir_lowering=False, debug=False)
    x = nc.dram_tensor("x", [NT, SEG, CW + 1], dt_io, kind="ExternalInput")
    w1 = nc.dram_tensor("w1", [SEG, SEG], dt_io, kind="ExternalInput")
    w2 = nc.dram_tensor("w2", [SEG, SEG], dt_io, kind="ExternalInput")
    y = nc.dram_tensor("y", [NT, SEG, CW], dt_io, kind="ExternalOutput")

    with tile.TileContext(nc) as tc:
        with (
            tc.tile_pool(name="const", bufs=1) as cpool,
            tc.tile_pool(name="xin", bufs=5) as xpool,
            tc.tile_pool(name="yout", bufs=4) as ypool,
            tc.tile_pool(name="ps", bufs=2, space="PSUM") as psum,
        ):
            # Weights go on the sync queue ahead of the input tiles (the
            # scalar queue's first issue sits behind a 1.3us ACT table
            # load). A tiny gpsimd-queue DMA warms that DGE ring early —
            # a fresh ring adds ~4us issue->packet latency to its first
            # DMA, which the output stream would otherwise pay.
            wt1 = cpool.tile([SEG, SEG], dt_io, tag="w1")
            wt2 = cpool.tile([SEG, SEG], dt_io, tag="w2")
            warm = cpool.tile([SEG, 16], dt_io, tag="warm")
            nc.gpsimd.dma_start(out=warm[:], in_=w1[:, 0:16])
            nc.sync.dma_start(out=wt1[:], in_=w1[:, :])
            nc.sync.dma_start(out=wt2[:], in_=w2[:, :])

            ncopy = 0
            for m in range(NT):
                xt = xpool.tile([SEG, CW + 1], dt_io, tag="xt")
                # Tile 0 loads in fine-grained segments so PE starts early.
                segw = [512, 512, 1024, 2048] if m == 0 else [CW]
                a_ = 0
                for w_ in segw:
                    b_ = a_ + w_ + (1 if a_ == 0 else 0)
                    nc.sync.dma_start(out=xt[:, a_:b_], in_=x[m, :, a_:b_])
                    a_ = b_

                yt = ypool.tile([SEG, CW], dt_io, tag="yt")
                # Per chunk: W1 pass, W2 pass (LDWEIGHTS double-buffers so
                # the swap hides), then a cast copy PSUM->SBUF and the
                # output DMA (gpsimd queue). Tile 0 uses fine chunks for an
                # early output start and the last tile for a short drain
                # tail; steady tiles use 2048-col chunks to cut instruction
                # and semaphore count (the copies must keep pace with the
                # 2.7us/tile input stream or the drain tail grows).
                if m == 0:
                    chunks = [512, 512, 1024, 1024, 1024]
                elif m == NT - 1:
                    chunks = [512] * 8
                else:
                    chunks = [2048, 2048]
                c0 = 0
                for cwid in chunks:
                    ptf = psum.tile([SEG, 2048], f32, tag="pt")
                    pt = ptf[:, :cwid]
                    for sub in range(cwid // GW or 1):
                        g0 = c0 + sub * GW
                        gw = min(GW, cwid)
                        nc.tensor.matmul(
                            pt[:, sub * gw : sub * gw + gw], wt1[:],
                            xt[:, 1 + g0 : 1 + g0 + gw],
                            start=True, stop=False)
                    for sub in range(cwid // GW or 1):
                        g0 = c0 + sub * GW
                        gw = min(GW, cwid)
                        nc.tensor.matmul(
                            pt[:, sub * gw : sub * gw + gw], wt2[:],
                            xt[:, g0 : g0 + gw],
                            start=False, stop=True)
                    # cast copy PSUM -> SBUF (gpsimd cannot read PSUM, so
                    # rotate scalar/vector only), then the output DMA on the
                    # copying engine's own queue (scalar) or gpsimd's: two
                    # queues drain the end-of-run output backlog in parallel.
                    dst = yt[:, c0 : c0 + cwid]
                    on_scalar = ncopy % 2 == 0
                    if on_scalar:
                        nc.scalar.copy(dst, pt[:])
                    else:
                        nc.vector.tensor_scalar_mul(dst, pt[:], 1.0)
                    ncopy += 1
                    c0 += cwid
                    qeng = nc.scalar if on_scalar else nc.gpsimd
                    qeng.dma_start(
                        out=y[m, :, c0 - cwid : c0], in_=yt[:, c0 - cwid : c0])

    nc.compile()
    return nc


_CACHE: dict = {}


def _get_program(mode):
    if mode not in _CACHE:
        _CACHE[mode] = _build_program(mode)
    return _CACHE[mode]


def _quant(arr, mode):
    """Cast host array to the device I/O dtype (numpy view of it)."""
    if mode == "fp16":
        return arr.astype(np.float16)
    if mode == "fp32r":
        return arr.astype(np.float32)
    if mode == "bf16":
        import ml_dtypes
        return arr.astype(ml_dtypes.bfloat16)
    raise ValueError(mode)


def _dequant(arr):
    return np.asarray(arr, dtype=np.float32)


def _weights(h, mode):
    J = len(h)
    W1 = np.zeros((SEG, SEG), np.float64)
    W2 = np.zeros((SEG, SEG), np.float64)
    for k in range(SEG):
        for i in range(SEG):
            d = i - k
            if 0 <= d < J:
                W1[k, i] = h[d]
            d2 = i + SEG - k
            if 1 <= d2 < J:
                W2[k, i] = h[d2]
    return _quant(W1, mode), _quant(W2, mode)


def run(x, b, a, trace: bool = False):
    """Run the kernel on the full (512, 65536) input; returns (y, exec_time_ns)."""
    from concourse.bass_utils import run_bass_kernel_spmd

    x = np.asarray(x, dtype=np.float32)
    assert x.shape == (BATCH, T), x.shape
    mode = IO_MODE
    h = _fir_taps(b, a)
    J = len(h)
    w1q, w2q = _weights(h, mode)
    nc = _get_program(mode)

    xq = _quant(x, mode)                       # (512, T) device-precision x
    in_maps = []
    for c in range(N_CORES):
        xc = xq[c * ROWS : (c + 1) * ROWS]     # (64, T)
        X = np.ascontiguousarray(
            xc.reshape(ROWS, F, SEG).transpose(2, 0, 1).reshape(SEG, COLS))
        Xp = np.concatenate([np.zeros((SEG, 1), X.dtype), X], axis=1)
        xd = np.stack([Xp[:, m * CW : m * CW + CW + 1] for m in range(NT)])
        in_maps.append({"x": xd, "w1": w1q, "w2": w2q})

    res = run_bass_kernel_spmd(nc, in_maps, list(range(N_CORES)), trace=trace)

    # Host-side fix of the one cross-row halo column per row: the device's
    # W2 matmul at each row's first group read the previous row's last
    # column. Subtract exactly what the device added (fp16 products are
    # exact in fp32, so recomputing with the quantized h/x matches PE).
    hq = _dequant(_quant(h, mode))             # (J,)
    NC_ = J - 1
    Hm = np.zeros((NC_, NC_), np.float32)      # Hm[d-1, i] = h[i+d]
    for dd in range(1, J):
        for i in range(0, J - dd):
            Hm[dd - 1, i] = hq[i + dd]

    out = np.empty((BATCH, T), dtype=np.float32)
    for c in range(N_CORES):
        yd = _dequant(res.results[c]["y"])     # (NT, SEG, CW)
        Y = yd.transpose(1, 0, 2).reshape(SEG, COLS)
        yc = np.ascontiguousarray(
            Y.reshape(SEG, ROWS, F).transpose(1, 2, 0).reshape(ROWS, T))
        xc = _dequant(xq[c * ROWS : (c + 1) * ROWS])
        # tail_rev[r, d-1] = x[r, T-d] for d in 1..J-1, rows 0..62
        tail_rev = xc[:-1, T - 1 : T - J : -1]           # (63, J-1)
        corr = tail_rev @ Hm                             # (63, J-1)
        yc[1:, : NC_] -= corr
        out[c * ROWS : (c + 1) * ROWS] = yc
    return out, res.exec_time_ns


def kernel(x, b, a):
    out, _ = run(x, b, a, trace=False)
    return out


# revision 20
# speedup vs baseline: 2.1139x; 1.0499x over previous
"""Trainium2 Bass kernel: batch biquad IIR as a truncated-FIR banded matmul.

The reference IIR y[t] = sum_m b[m] x[t-m]/a0 - sum_n a[n]/a0 y[t-n] has a
fast-decaying impulse response for this filter (poles at |z| = sqrt(0.1716)),
so y is computed exactly (to below-fp32-noise truncation, |h| < 1e-9) as a
J~25-tap FIR of x. This removes the time recurrence entirely: no DVE scan
(3 cycles/element serial — 82us/core for this shape), no feedback state.

Layout: each waveform row (65536 samples) is reshaped host-side to a
[128, 512] segment matrix X (partition k holds samples 128*f + k). Then

    y(128f + i) = sum_k W1[k, i] X[k, f] + sum_k W2[k, i] X[k, f-1]

with W1[k,i] = h[i-k] (banded lower Toeplitz) and W2[k,i] = h[i+128-k]
(corner band), i.e. TWO accumulating PE matmuls per 512-column group with
stationary weights. Rows are concatenated along the free axis; the one
cross-row halo column per row start is corrected on the host (the leak is
linear and only touches the first J-1 samples of each row).

Engine budget per core (8 tiles x 4096 cols): PE ~30us of fp16 matmuls
(1 cyc/col), PSUM->SBUF cast copies split across Scalar/DVE/Pool ~15us
each, DMA 2x8.4MB fp16 ~47us -> DMA-bound.

Sharding: pure data parallel, 64 rows per core on 8 cores. I/O is fp16
(x cast host-side, y cast back): halves HBM traffic; error ~1e-3 absolute
(~2.5e-4 of output scale) vs the 2e-2 harness gate.
"""

import numpy as np

# Problem geometry (hardcoded per the grading contract).
N_CORES = 8
BATCH = 512
T = 65536
ROWS = BATCH // N_CORES          # 64 rows per core
SEG = 128                        # samples per segment column (= partitions)
F = T // SEG                     # 512 segment columns per row
COLS = ROWS * F                  # 32768 free-axis columns per core
CW = 4096                        # tile width (free cols); 8 groups of 512
NT = COLS // CW                  # 8 tiles
GW = 512                         # matmul group cols (one PSUM bank)
HW_ = 1024                       # copy granularity (two banks)

# I/O + matmul precision mode: "fp16" | "bf16" | "fp32r"
IO_MODE = "fp16"


def _fir_taps(b, a, tol=1e-9, max_taps=120):
    """Impulse response of the IIR filter, truncated to J taps (float64)."""
    b = np.asarray(b, np.float64)
    a = np.asarray(a, np.float64)
    L = 256
    h = np.zeros(L)
    for t in range(L):
        acc = b[t] if t < len(b) else 0.0
        for n in range(1, len(a)):
            if t - n >= 0:
                acc -= a[n] * h[t - n]
        h[t] = acc / a[0]
    mag = np.abs(h)
    keep = np.nonzero(mag > tol * mag.max())[0]
    J = int(keep.max()) + 1
    assert J <= max_taps, f"impulse response too long for FIR approach: {J}"
    assert J <= SEG, J
    return h[:J]


def _np_dt(mode):
    return {"fp16": np.float16, "bf16": None, "fp32r": np.float32}[mode]


def _build_program(mode):
    import concourse.bacc as bacc
    import concourse.mybir as mybir
    import concourse.tile as tile

    dt_io = {
        "fp16": mybir.dt.float16,
        "bf16": mybir.dt.bfloat16,
        "fp32r": mybir.dt.float32r,
    }[mode]
    f32 = mybir.dt.float32

    nc = bacc.Bacc("TRN2", target_bir_lowering=False, debug=False)
    x = nc.dram_tensor("x", [NT, SEG, CW + 1], dt_io, kind="ExternalInput")
    w1 = nc.dram_tensor("w1", [SEG, SEG], dt_io, kind="ExternalInput")
    w2 = nc.dram_tensor("w2", [SEG, SEG], dt_io, kind="ExternalInput")
    y = nc.dram_tensor("y", [NT, SEG, CW], dt_io, kind="ExternalOutput")

    with tile.TileContext(nc) as tc:
        with (
            tc.tile_pool(name="const", bufs=1) as cpool,
            tc.tile_pool(name="xin", bufs=5) as xpool,
            tc.tile_pool(name="yout", bufs=4) as ypool,
            tc.tile_pool(name="ps", bufs=4, space="PSUM") as psum,
        ):
            # Weights go on the sync queue ahead of the input tiles (the
            # scalar queue's first issue sits behind a 1.3us ACT table
            # load). A tiny gpsimd-queue DMA warms that DGE ring early —
            # a fresh ring adds ~4us issue->packet latency to its first
            # DMA, which the output stream would otherwise pay.
            wt1 = cpool.tile([SEG, SEG], dt_io, tag="w1")
            wt2 = cpool.tile([SEG, SEG], dt_io, tag="w2")
            warm = cpool.tile([SEG, 16], dt_io, tag="warm")
            nc.gpsimd.dma_start(out=warm[:], in_=w1[:, 0:16])
            nc.sync.dma_start(out=wt1[:], in_=w1[:, :])
            nc.sync.dma_start(out=wt2[:], in_=w2[:, :])

            ncopy = 0
            for m in range(NT):
                xt = xpool.tile([SEG, CW + 1], dt_io, tag="xt")
                # Tile 0 loads in fine-grained segments so PE starts early.
                segw = [512, 512, 1024, 2048] if m == 0 else [CW]
                a_ = 0
                for w_ in segw:
                    b_ = a_ + w_ + (1 if a_ == 0 else 0)
                    nc.sync.dma_start(out=xt[:, a_:b_], in_=x[m, :, a_:b_])
                    a_ = b_

                yt = ypool.tile([SEG, CW], dt_io, tag="yt")
                # Per chunk: W1 pass, W2 pass (LDWEIGHTS double-buffers so
                # the swap hides), then a cast copy PSUM->SBUF and the
                # output DMA (gpsimd queue). Tile 0 uses fine chunks for an
                # early output start and the last tile for a short drain
                # tail; steady tiles use 2048-col chunks to cut instruction
                # and semaphore count (the copies must keep pace with the
                # 2.7us/tile input stream or the drain tail grows).
                if m == 0:
                    chunks = [512, 512, 1024, 1024, 1024]
                elif m == NT - 1:
                    chunks = [512] * 8
                else:
                    chunks = [1024, 1024, 1024, 1024]
                c0 = 0
                for cwid in chunks:
                    ptf = psum.tile([SEG, 1024], f32, tag="pt")
                    pt = ptf[:, :cwid]
                    for sub in range(cwid // GW or 1):
                        g0 = c0 + sub * GW
                        gw = min(GW, cwid)
                        nc.tensor.matmul(
                            pt[:, sub * gw : sub * gw + gw], wt1[:],
                            xt[:, 1 + g0 : 1 + g0 + gw],
                            start=True, stop=False)
                    for sub in range(cwid // GW or 1):
                        g0 = c0 + sub * GW
                        gw = min(GW, cwid)
                        nc.tensor.matmul(
                            pt[:, sub * gw : sub * gw + gw], wt2[:],
                            xt[:, g0 : g0 + gw],
                            start=False, stop=True)
                    # cast copy PSUM -> SBUF (gpsimd cannot read PSUM, so
                    # rotate scalar/vector only), then the output DMA on the
                    # copying engine's own queue (scalar) or gpsimd's: two
                    # queues drain the end-of-run output backlog in parallel.
                    dst = yt[:, c0 : c0 + cwid]
                    on_scalar = ncopy % 2 == 0
                    if on_scalar:
                        nc.scalar.copy(dst, pt[:])
                    else:
                        nc.vector.tensor_scalar_mul(dst, pt[:], 1.0)
                    ncopy += 1
                    c0 += cwid
                    qeng = nc.scalar if on_scalar else nc.gpsimd
                    qeng.dma_start(
                        out=y[m, :, c0 - cwid : c0], in_=yt[:, c0 - cwid : c0])

    nc.compile()
    return nc


_CACHE: dict = {}


def _get_program(mode):
    if mode not in _CACHE:
        _CACHE[mode] = _build_program(mode)
    return _CACHE[mode]


def _quant(arr, mode):
    """Cast host array to the device I/O dtype (numpy view of it)."""
    if mode == "fp16":
        return arr.astype(np.float16)
    if mode == "fp32r":
        return arr.astype(np.float32)
    if mode == "bf16":
        import ml_dtypes
        return arr.astype(ml_dtypes.bfloat16)
    raise ValueError(mode)


def _dequant(arr):
    return np.asarray(arr, dtype=np.float32)


def _weights(h, mode):
    J = len(h)
    W1 = np.zeros((SEG, SEG), np.float64)
    W2 = np.zeros((SEG, SEG), np.float64)
    for k in range(SEG):
        for i in range(SEG):
            d = i - k
            if 0 <= d < J:
                W1[k, i] = h[d]
            d2 = i + SEG - k
            if 1 <= d2 < J:
                W2[k, i] = h[d2]
    return _quant(W1, mode), _quant(W2, mode)


def run(x, b, a, trace: bool = False):
    """Run the kernel on the full (512, 65536) input; returns (y, exec_time_ns)."""
    from concourse.bass_utils import run_bass_kernel_spmd

    x = np.asarray(x, dtype=np.float32)
    assert x.shape == (BATCH, T), x.shape
    mode = IO_MODE
    h = _fir_taps(b, a)
    J = len(h)
    w1q, w2q = _weights(h, mode)
    nc = _get_program(mode)

    xq = _quant(x, mode)                       # (512, T) device-precision x
    in_maps = []
    for c in range(N_CORES):
        xc = xq[c * ROWS : (c + 1) * ROWS]     # (64, T)
        X = np.ascontiguousarray(
            xc.reshape(ROWS, F, SEG).transpose(2, 0, 1).reshape(SEG, COLS))
        Xp = np.concatenate([np.zeros((SEG, 1), X.dtype), X], axis=1)
        xd = np.stack([Xp[:, m * CW : m * CW + CW + 1] for m in range(NT)])
        in_maps.append({"x": xd, "w1": w1q, "w2": w2q})

    res = run_bass_kernel_spmd(nc, in_maps, list(range(N_CORES)), trace=trace)

    # Host-side fix of the one cross-row halo column per row: the device's
    # W2 matmul at each row's first group read the previous row's last
    # column. Subtract exactly what the device added (fp16 products are
    # exact in fp32, so recomputing with the quantized h/x matches PE).
    hq = _dequant(_quant(h, mode))             # (J,)
    NC_ = J - 1
    Hm = np.zeros((NC_, NC_), np.float32)      # Hm[d-1, i] = h[i+d]
    for dd in range(1, J):
        for i in range(0, J - dd):
            Hm[dd - 1, i] = hq[i + dd]

    out = np.empty((BATCH, T), dtype=np.float32)
    for c in range(N_CORES):
        yd = _dequant(res.results[c]["y"])     # (NT, SEG, CW)
        Y = yd.transpose(1, 0, 2).reshape(SEG, COLS)
        yc = np.ascontiguousarray(
            Y.reshape(SEG, ROWS, F).transpose(1, 2, 0).reshape(ROWS, T))
        xc = _dequant(xq[c * ROWS : (c + 1) * ROWS])
        # tail_rev[r, d-1] = x[r, T-d] for d in 1..J-1, rows 0..62
        tail_rev = xc[:-1, T - 1 : T - J : -1]           # (63, J-1)
        corr = tail_rev @ Hm                             # (63, J-1)
        yc[1:, : NC_] -= corr
        out[c * ROWS : (c + 1) * ROWS] = yc
    return out, res.exec_time_ns


def kernel(x, b, a):
    out, _ = run(x, b, a, trace=False)
    return out
